# revision 45
# baseline (speedup 1.0000x reference)
"""AdaptiveConv Trainium2 kernel — 8-core SPMD, batch-sharded.

Per full batch:
  x [16, 256, 64, 64] f32, w [16, 512] f32,
  filter_bank [8, 256, 256, 3, 3], dense_fw (512->8), dense_mod (512->256).
  fbw = softmax(w @ Wfw + bfw)                  [16, 8]
  filters = einsum('bfchw,nb->nfchw', bank, fbw)
  filters *= (w @ Wmod + bmod + 1)[n, f]
  norm[n,kh,kw] = sqrt(max(sum_{f,c} filters^2, 1e-8)); filters /= norm
  out[n] = conv2d_same(x[n], filters[n])        [16, 256, 64, 64]

Sharding: batch N=16 over 8 cores (2 samples each); small params + the
filter bank replicated. The conv is an implicit GEMM in padded-flat
pixel coordinates: x lives in SBUF as [c=128, 66*66] zero-halo tiles,
so each of the 18 accumulating matmuls (2 c-tiles x 9 taps) per output
chunk streams a fully contiguous 512-wide rhs (full PE rate). Output
columns 64,65 of each row are garbage and dropped by the strided
output DMA.

Filter mixing uses DVE fast modes (tensor_scalar 4x + tensor_tensor
2x in bf16) instead of 1x scalar_tensor_tensor. Dummy paced matmuls
keep the PE HAM clock warm while sample 0's filters are being mixed.

Host-side work is layout-only: batch slicing, a transpose of
filter_bank to [b, c, tap, f] (+ optional bf16 cast), and w to
[p, ko, n] so every DMA is a single contiguous descriptor run.
"""

import os
import sys

import numpy as np

if "/opt/trn_rl_repo" not in sys.path:
    sys.path.insert(0, "/opt/trn_rl_repo")

import concourse.bacc as bacc_mod
import concourse.mybir as mybir
import concourse.tile as tile
from concourse.bass_utils import run_bass_kernel_spmd

N_CORES = 8
NS = 2            # samples per core
C = 256
F = 256
H = W = 64
KK = 3
TAPS = KK * KK    # 9
NF = 8
WD = 512
P = 128
CT = C // P       # 2 c tiles
FT = F // P       # 2 f tiles
KO = WD // P      # 4 contraction tiles for the dense layers
HP = H + 2        # 66
WP = W + 2        # 66
XL = HP * WP      # 4356 padded pixels
XLPAD = XL + 8    # tile length incl. slack for tap-offset over-reads
OPIX = H * WP     # 4224 output pixels in padded-w coords
CHUNKS = [(i * 512, 512) for i in range(8)] + [(4096, OPIX - 4096)]
MIX_CHUNKS = [(i * 512, 512) for i in range(4)] + [(2048, TAPS * F - 2048)]
EPS = 1e-8

USE_BF16 = os.environ.get("KERNEL_F32", "") != "1"
WARM_MM = int(os.environ.get("KERNEL_WARM_MM", "2"))  # warmup MMs per bank DMA

LAST = None       # BassKernelResults of the most recent run (for test.py)


def _build():
    f32 = mybir.dt.float32
    cdt = mybir.dt.bfloat16 if USE_BF16 else f32

    nc = bacc_mod.Bacc()
    x_d = nc.declare_dram_parameter("xp", [NS, CT, P, XLPAD], cdt, isOutput=False)
    w_d = nc.declare_dram_parameter("wv_t", [P, KO, NS], f32, isOutput=False)
    bank_d = nc.declare_dram_parameter("bank_t", [NF, C, TAPS, F], cdt,
                                       isOutput=False)
    fww_d = nc.declare_dram_parameter("fw_w", [WD, NF], cdt, isOutput=False)
    fwb_d = nc.declare_dram_parameter("fw_b", [NF], f32, isOutput=False)
    mdw_d = nc.declare_dram_parameter("md_w", [WD, F], cdt, isOutput=False)
    mdb_d = nc.declare_dram_parameter("md_b", [F], f32, isOutput=False)
    ident_d = nc.declare_dram_parameter("ident", [P, P], cdt, isOutput=False)
    out_d = nc.declare_dram_parameter("out", [NS, F, H, W], f32, isOutput=True)

    mm = mybir.AluOpType.mult
    aa = mybir.AluOpType.add
    ACT = mybir.ActivationFunctionType

    with tile.TileContext(nc) as tc, \
         tc.tile_pool(name="const", bufs=1) as const_p, \
         tc.tile_pool(name="small", bufs=2) as small_p, \
         tc.tile_pool(name="bcast", bufs=2) as bc_p, \
         tc.tile_pool(name="dscr", bufs=1, space="DRAM") as dram_p, \
         tc.tile_pool(name="xpad", bufs=(4 if USE_BF16 else 3)) as xpad_p, \
         tc.tile_pool(name="bank", bufs=(8 if USE_BF16 else 2)) as bank_p, \
         tc.tile_pool(name="tmp", bufs=3) as tmp_p, \
         tc.tile_pool(name="diag", bufs=8) as diag_p, \
         tc.tile_pool(name="mix", bufs=2) as mix_p, \
         tc.tile_pool(name="filtb", bufs=3) as filtb_p, \
         tc.tile_pool(name="outs", bufs=2) as out_p, \
         tc.tile_pool(name="pscv", bufs=6, space="PSUM") as ps_conv, \
         tc.tile_pool(name="pssm", bufs=1, space="PSUM") as ps_small:

        # preload ACT tables (Exp/Square/Sqrt) with a self-contained tile
        # so the loads never join the softmax dependency chain
        tblw = small_p.tile([P, 2], f32, name="tblw")
        nc.vector.memset(tblw, 0.5)
        nc.scalar.activation(tblw, tblw, ACT.Exp)
        nc.scalar.activation(tblw, tblw, ACT.Square)
        nc.scalar.activation(tblw, tblw, ACT.Sqrt)

        # ---- small parameter loads -------------------------------------
        wt = const_p.tile([P, KO, NS], f32)
        nc.sync.dma_start(wt, w_d[:, :, :])
        fww = const_p.tile([P, KO, NF], cdt)
        nc.sync.dma_start(fww, fww_d[:, :].rearrange("(ko p) f -> p ko f", p=P))
        mdw = const_p.tile([P, KO, F], cdt)
        nc.scalar.dma_start(mdw, mdw_d[:, :].rearrange("(ko p) f -> p ko f", p=P))
        fwb_bc = const_p.tile([P, NF], f32)
        nc.scalar.dma_start(fwb_bc, fwb_d[:][None, :].to_broadcast((P, NF)))
        mdb_bc = const_p.tile([P, F], f32)
        nc.scalar.dma_start(mdb_bc, mdb_d[:][None, :].to_broadcast((P, F)))


        # ---- dense heads, replicated across all 128 partitions ---------
        # lhsT columns all equal w[n], so every psum partition holds the
        # same dense result; softmax runs redundantly per partition and
        # fbw/w1 land pre-broadcast with no DRAM bounce.
        fbw_bc = []
        w1rep = []
        for n in range(NS):
            wrep = bc_p.tile([P, KO, P], cdt, tag="wrep")
            nc.vector.tensor_copy(
                wrep, wt[:, :, n][:, :, None].to_broadcast((P, KO, P)))
            ps_l = ps_small.tile([P, NF], f32, tag="sm")
            for ko in range(KO):
                nc.tensor.matmul(ps_l, lhsT=wrep[:, ko, :], rhs=fww[:, ko, :],
                                 start=(ko == 0), stop=(ko == KO - 1))
            logits = small_p.tile([P, NF], f32, tag="logits")
            nc.vector.tensor_tensor(logits, ps_l, fwb_bc, aa)
            # softmax denominator cancels through the per-tap demod norm
            # (filters/||filters|| is invariant to any per-sample scalar),
            # so raw exp(logits) works as mixing weights.
            fb = bc_p.tile([P, NF], f32, tag="fbw_bc")
            nc.scalar.activation(fb, logits, ACT.Exp)
            fbw_bc.append(fb)

            ps_sc = ps_small.tile([P, F], f32, tag="sm")
            for ko in range(KO):
                nc.tensor.matmul(ps_sc, lhsT=wrep[:, ko, :], rhs=mdw[:, ko, :],
                                 start=(ko == 0), stop=(ko == KO - 1))
            w1 = bc_p.tile([P, F], cdt, tag="w1_bc")
            nc.vector.scalar_tensor_tensor(w1, ps_sc, 1.0, mdb_bc, aa, aa)
            w1r = bc_p.tile([P, TAPS, F], cdt, tag="w1rep")
            nc.vector.tensor_copy(
                w1r, w1[:, None, :].to_broadcast((P, TAPS, F)))
            w1rep.append(w1r)

        # psum for HAM warmup matmuls
        ps_warm = ps_small.tile([P, 512], f32, tag="warm")
        ones_sb = const_p.tile([P, P], f32)
        nc.vector.memset(ones_sb, 1.0)
        ident = const_p.tile([P, P], cdt)
        nc.scalar.dma_start(ident, ident_d[:, :])

        # ---- per-sample: pad x, mix+modulate+demod filters, conv -------
        for n in range(NS):
            # host-pre-padded input in flat coords, [c=128, 66*66+slack]
            xpt = []
            for ct in range(CT):
                xp = xpad_p.tile([P, XLPAD], cdt, tag="xpad")
                nc.scalar.dma_start(xp, x_d[n, ct, :, :])
                xpt.append(xp)

            # mix bank with fbw, then modulate by w1.
            # Sample 0 is on the critical path: accumulate fbw_b * bank_b on
            # the otherwise-idle PE as matmuls with diagonal fbw_b*I weights
            # (f32 PSUM accumulate). Sample 1 mixes on DVE under sample 0's
            # conv.
            acc = []
            q0 = []
            if n == 0:
                fbwI = []
                for b in range(NF):
                    dg = diag_p.tile([P, P], cdt, tag="diag")
                    nc.vector.tensor_scalar_mul(dg, ident,
                                                fbw_bc[n][:, b:b + 1])
                    fbwI.append(dg)
                    if WARM_MM:
                        for _ in range(2):
                            nc.tensor.matmul(ps_warm[:, :P], lhsT=dg, rhs=dg,
                                             start=True, stop=True)
                for ct in range(CT):
                    bks = []
                    for b in range(NF):
                        bk = bank_p.tile([P, TAPS, F], cdt, tag=f"bk{ct}")
                        nc.sync.dma_start(bk,
                                          bank_d[b, ct * P:(ct + 1) * P, :, :])
                        bks.append(bk)
                        if ct == 0 and WARM_MM:
                            for _ in range(WARM_MM):
                                nc.tensor.matmul(ps_warm,
                                                 lhsT=bk[:, 0, 0:P],
                                                 rhs=bk[:, 0:2, :],
                                                 start=True, stop=True)
                    a = mix_p.tile([P, TAPS, F], cdt, tag="acc")
                    af = a.rearrange("p t f -> p (t f)")
                    scr = tmp_p.tile([P, TAPS, F], cdt, tag="tmp")
                    scrf = scr.rearrange("p t f -> p (t f)")
                    w1f = w1rep[n].rearrange("p t f -> p (t f)")
                    qt = small_p.tile([P, TAPS], f32, tag="q")
                    for ci, (off, csz) in enumerate(MIX_CHUNKS):
                        ps = ps_conv.tile([P, 512], f32, tag="cv")
                        for b in range(NF):
                            bf = bks[b].rearrange("p t f -> p (t f)")
                            nc.tensor.matmul(ps[:, :csz], lhsT=fbwI[b],
                                             rhs=bf[:, off:off + csz],
                                             start=(b == 0),
                                             stop=(b == NF - 1))
                        nc.vector.tensor_tensor(af[:, off:off + csz],
                                                ps[:, :csz],
                                                w1f[:, off:off + csz], mm)
                        nc.scalar.activation(scrf[:, off:off + csz],
                                             af[:, off:off + csz], ACT.Square)
                        if ci == 3:
                            # taps 0-7 reduce overlaps the final mix chunk
                            nc.vector.tensor_reduce(
                                qt[:, 0:8], scr[:, 0:8, :],
                                axis=mybir.AxisListType.X, op=aa)
                    nc.vector.tensor_reduce(qt[:, 8:9], scr[:, 8:9, :],
                                            axis=mybir.AxisListType.X, op=aa)
                    q0.append(qt)
                    acc.append(a)
            else:
                for ct in range(CT):
                    a = mix_p.tile([P, TAPS, F], cdt, tag="acc")
                    for b in range(NF):
                        bk = bank_p.tile([P, TAPS, F], cdt, tag=f"bk{ct}")
                        nc.sync.dma_start(bk,
                                          bank_d[b, ct * P:(ct + 1) * P, :, :])
                        if b == 0:
                            nc.vector.tensor_scalar_mul(a, bk,
                                                        fbw_bc[n][:, 0:1])
                        else:
                            t = tmp_p.tile([P, TAPS, F], cdt, tag="tmp")
                            if b in (5, 7):
                                nc.scalar.activation(
                                    t, bk, ACT.Copy,
                                    scale=fbw_bc[n][:, b:b + 1])
                            else:
                                nc.vector.tensor_scalar_mul(
                                    t, bk, fbw_bc[n][:, b:b + 1])
                            nc.vector.tensor_tensor(a, a, t, aa)
                    nc.vector.tensor_tensor(a, a, w1rep[n], mm)
                    acc.append(a)

            # per-tap demod norm over (f, c)
            if n == 0:
                q = q0
            else:
                q = []
                for ct in range(CT):
                    scr = tmp_p.tile([P, TAPS, F], cdt, tag="tmp")
                    nc.scalar.activation(scr, acc[ct], ACT.Square)
                    qt = small_p.tile([P, TAPS], f32, tag="q")
                    nc.vector.tensor_reduce(qt, scr,
                                            axis=mybir.AxisListType.X, op=aa)
                    q.append(qt)
            qs = small_p.tile([P, TAPS], f32, tag="qs")
            nc.vector.tensor_tensor(qs, q[0], q[1], aa)
            ps_nrm = ps_small.tile([P, TAPS], f32, tag="sm")
            nc.tensor.matmul(ps_nrm, lhsT=ones_sb, rhs=qs,
                             start=True, stop=True)
            nall = small_p.tile([P, TAPS], f32, tag="nall")
            nc.vector.tensor_scalar_max(nall, ps_nrm, EPS)
            sq = small_p.tile([P, TAPS], f32, tag="sq")
            nc.scalar.activation(sq, nall, ACT.Sqrt)
            ninv = small_p.tile([P, TAPS], f32, tag="ninv")
            nc.vector.reciprocal(ninv, sq)
            if n == 0 and WARM_MM:
                for _ in range(4):
                    nc.tensor.matmul(ps_warm[:, :TAPS], lhsT=ones_sb,
                                     rhs=qs, start=True, stop=True)

            filt = []
            for ct in range(CT):
                fl = filtb_p.tile([P, TAPS, F], cdt, tag="filt")
                for tp in range(TAPS):
                    if tp % 2 == 0:
                        nc.vector.tensor_scalar_mul(
                            fl[:, tp, :], acc[ct][:, tp, :],
                            ninv[:, tp:tp + 1])
                    else:
                        nc.scalar.activation(fl[:, tp, :], acc[ct][:, tp, :],
                                             ACT.Copy,
                                             scale=ninv[:, tp:tp + 1])
                filt.append(fl)

            # conv in padded-flat coords: rhs slices are contiguous so the
            # PE streams at full rate; cols 64,65 of each row are garbage
            # and dropped by the strided output DMA.
            for ft in range(FT):
                osb = out_p.tile([P, OPIX], f32, tag="osb")
                for off, sz in CHUNKS:
                    ps = ps_conv.tile([P, 512], f32, tag="cv")
                    k = 0
                    for ct in range(CT):
                        for kh in range(KK):
                            for kw in range(KK):
                                rhs = xpt[ct][:, off + kh * WP + kw:
                                              off + kh * WP + kw + sz]
                                nc.tensor.matmul(
                                    ps[:, :sz],
                                    lhsT=filt[ct][:, kh * KK + kw,
                                                  ft * P:(ft + 1) * P],
                                    rhs=rhs,
                                    start=(k == 0), stop=(k == 2 * TAPS - 1))
                                k += 1
                    nc.scalar.activation(osb[:, off:off + sz], ps[:, :sz],
                                         ACT.Copy)
                ov = osb.rearrange("p (h w) -> p h w", w=WP)
                HS = 38
                nc.scalar.dma_start(out_d[n, ft * P:(ft + 1) * P, 0:HS, :],
                                    ov[:, 0:HS, 0:W])
                nc.scalar.dma_start(out_d[n, ft * P:(ft + 1) * P, HS:H, :],
                                    ov[:, HS:H, 0:W])

    nc.compile()
    return nc


def kernel(x, w, filter_bank, dense_fw_w, dense_fw_b, dense_mod_w, dense_mod_b):
    global LAST
    x = np.ascontiguousarray(np.asarray(x, dtype=np.float32))
    w = np.ascontiguousarray(np.asarray(w, dtype=np.float32))
    xdt = np.float32
    if USE_BF16:
        import ml_dtypes
        xdt = ml_dtypes.bfloat16
    NB = x.shape[0]
    xp_all = np.zeros((NB, CT, P, XLPAD), dtype=xdt)
    xv = xp_all[:, :, :, :XL].reshape(NB, CT, P, HP, WP)
    xv[:, :, :, 1:H + 1, 1:W + 1] = x.reshape(NB, CT, P, H, W)
    fb = np.asarray(filter_bank, dtype=np.float32)
    # [b, f, c, kh, kw] -> [b, c, (kh kw), f]
    bank_t = np.ascontiguousarray(
        np.transpose(fb, (0, 2, 3, 4, 1)).reshape(NF, C, TAPS, F))
    if USE_BF16:
        import ml_dtypes
        bank_t = bank_t.astype(ml_dtypes.bfloat16)

    trace = os.environ.get("KERNEL_TRACE", "") == "1"
    if trace:
        import types

        import concourse.bass_utils as bu
        bu.upload_artifacts = lambda tmpdir: tmpdir
        if "antenv.axon_hooks" not in sys.modules:
            from trn_agent_boot.trn_boot import _ntff_profile_via_ctypes
            hook = _ntff_profile_via_ctypes("/opt/axon/libaxon_pjrt.so")
            mod = types.ModuleType("antenv.axon_hooks")
            mod.get_axon_ntff_profile_hook = lambda: hook
            sys.modules["antenv.axon_hooks"] = mod

    nc = _build()
    in_maps = []
    for core in range(N_CORES):
        sl = slice(core * NS, (core + 1) * NS)
        w_t = np.ascontiguousarray(
            w[sl].reshape(NS, KO, P).transpose(2, 1, 0))
        ident = np.eye(P, dtype=xdt)
        in_maps.append({
            "ident": ident,
            "xp": np.ascontiguousarray(xp_all[sl]),
            "wv_t": w_t,
            "bank_t": bank_t,
            "fw_w": np.ascontiguousarray(np.asarray(dense_fw_w, np.float32).astype(xdt)),
            "fw_b": np.ascontiguousarray(np.asarray(dense_fw_b, np.float32)),
            "md_w": np.ascontiguousarray(np.asarray(dense_mod_w, np.float32).astype(xdt)),
            "md_b": np.ascontiguousarray(np.asarray(dense_mod_b, np.float32)),
        })
    kwargs = {}
    if trace:
        import tempfile
        base = os.environ.get("KERNEL_TRACE_DIR", "/tmp/ktrace")
        os.makedirs(base, exist_ok=True)
        tdir = tempfile.mkdtemp(dir=base)
        print(f"trace dir: {tdir}", flush=True)
        kwargs = dict(trace=True, tmpdir=tdir)
    LAST = run_bass_kernel_spmd(nc, in_maps, core_ids=list(range(N_CORES)),
                                **kwargs)
    return np.concatenate([LAST.results[i]["out"] for i in range(N_CORES)],
                          axis=0)


# revision 46
# speedup vs baseline: 1.1699x; 1.1699x over previous
"""AdaptiveConv Trainium2 kernel — 8-core SPMD, batch-sharded.

Per full batch:
  x [16, 256, 64, 64] f32, w [16, 512] f32,
  filter_bank [8, 256, 256, 3, 3], dense_fw (512->8), dense_mod (512->256).
  fbw = softmax(w @ Wfw + bfw)                  [16, 8]
  filters = einsum('bfchw,nb->nfchw', bank, fbw)
  filters *= (w @ Wmod + bmod + 1)[n, f]
  norm[n,kh,kw] = sqrt(max(sum_{f,c} filters^2, 1e-8)); filters /= norm
  out[n] = conv2d_same(x[n], filters[n])        [16, 256, 64, 64]

Sharding: batch N=16 over 8 cores (2 samples each); small params + the
filter bank replicated. The conv is an implicit GEMM in padded-flat
pixel coordinates: x lives in SBUF as [c=128, 66*66] zero-halo tiles,
so each of the 18 accumulating matmuls (2 c-tiles x 9 taps) per output
chunk streams a fully contiguous 512-wide rhs (full PE rate). Output
columns 64,65 of each row are garbage and dropped by the strided
output DMA.

Filter mixing uses DVE fast modes (tensor_scalar 4x + tensor_tensor
2x in bf16) instead of 1x scalar_tensor_tensor. Dummy paced matmuls
keep the PE HAM clock warm while sample 0's filters are being mixed.

Host-side work is layout-only: batch slicing, a transpose of
filter_bank to [b, c, tap, f] (+ optional bf16 cast), and w to
[p, ko, n] so every DMA is a single contiguous descriptor run.
"""

import os
import sys

import numpy as np

if "/opt/trn_rl_repo" not in sys.path:
    sys.path.insert(0, "/opt/trn_rl_repo")

import concourse.bacc as bacc_mod
import concourse.mybir as mybir
import concourse.tile as tile
from concourse.bass_utils import run_bass_kernel_spmd

N_CORES = 8
NS = 2            # samples per core
C = 256
F = 256
H = W = 64
KK = 3
TAPS = KK * KK    # 9
NF = 8
WD = 512
P = 128
CT = C // P       # 2 c tiles
FT = F // P       # 2 f tiles
KO = WD // P      # 4 contraction tiles for the dense layers
HP = H + 2        # 66
WP = W + 2        # 66
XL = HP * WP      # 4356 padded pixels
XLPAD = XL + 8    # tile length incl. slack for tap-offset over-reads
OPIX = H * WP     # 4224 output pixels in padded-w coords
CHUNKS = [(i * 512, 512) for i in range(8)] + [(4096, OPIX - 4096)]
MIX_CHUNKS = [(i * 512, 512) for i in range(4)] + [(2048, TAPS * F - 2048)]
EPS = 1e-8

USE_BF16 = os.environ.get("KERNEL_F32", "") != "1"
WARM_MM = int(os.environ.get("KERNEL_WARM_MM", "2"))  # warmup MMs per bank DMA

LAST = None       # BassKernelResults of the most recent run (for test.py)


def _build():
    f32 = mybir.dt.float32
    cdt = mybir.dt.bfloat16 if USE_BF16 else f32

    nc = bacc_mod.Bacc()
    x_d = nc.declare_dram_parameter("xp", [NS, CT, P, XLPAD], cdt, isOutput=False)
    w_d = nc.declare_dram_parameter("wv_t", [P, KO, NS], f32, isOutput=False)
    bank_d = nc.declare_dram_parameter("bank_t", [NF, C, TAPS, F], cdt,
                                       isOutput=False)
    fww_d = nc.declare_dram_parameter("fw_w", [WD, NF], cdt, isOutput=False)
    fwb_d = nc.declare_dram_parameter("fw_b", [NF], f32, isOutput=False)
    mdw_d = nc.declare_dram_parameter("md_w", [WD, F], cdt, isOutput=False)
    mdb_d = nc.declare_dram_parameter("md_b", [F], f32, isOutput=False)
    ident_d = nc.declare_dram_parameter("ident", [P, P], cdt, isOutput=False)
    out_d = nc.declare_dram_parameter("out", [NS, F, H, W], f32, isOutput=True)

    mm = mybir.AluOpType.mult
    aa = mybir.AluOpType.add
    ACT = mybir.ActivationFunctionType

    with tile.TileContext(nc) as tc, \
         tc.tile_pool(name="const", bufs=1) as const_p, \
         tc.tile_pool(name="small", bufs=2) as small_p, \
         tc.tile_pool(name="bcast", bufs=2) as bc_p, \
         tc.tile_pool(name="dscr", bufs=1, space="DRAM") as dram_p, \
         tc.tile_pool(name="xpad", bufs=(4 if USE_BF16 else 3)) as xpad_p, \
         tc.tile_pool(name="bank", bufs=(8 if USE_BF16 else 2)) as bank_p, \
         tc.tile_pool(name="tmp", bufs=3) as tmp_p, \
         tc.tile_pool(name="diag", bufs=8) as diag_p, \
         tc.tile_pool(name="mix", bufs=2) as mix_p, \
         tc.tile_pool(name="filtb", bufs=3) as filtb_p, \
         tc.tile_pool(name="outs", bufs=2) as out_p, \
         tc.tile_pool(name="pscv", bufs=6, space="PSUM") as ps_conv, \
         tc.tile_pool(name="pssm", bufs=1, space="PSUM") as ps_small:

        # preload ACT tables (Exp/Square/Sqrt) with a self-contained tile
        # so the loads never join the softmax dependency chain
        tblw = small_p.tile([P, 2], f32, name="tblw")
        nc.vector.memset(tblw, 0.5)
        nc.scalar.activation(tblw, tblw, ACT.Exp)
        nc.scalar.activation(tblw, tblw, ACT.Square)
        nc.scalar.activation(tblw, tblw, ACT.Sqrt)

        # ---- small parameter loads -------------------------------------
        wt = const_p.tile([P, KO, NS], f32)
        nc.sync.dma_start(wt, w_d[:, :, :])
        fww = const_p.tile([P, KO, NF], cdt)
        nc.sync.dma_start(fww, fww_d[:, :].rearrange("(ko p) f -> p ko f", p=P))
        mdw = const_p.tile([P, KO, F], cdt)
        nc.sync.dma_start(mdw, mdw_d[:, :].rearrange("(ko p) f -> p ko f", p=P))
        fwb_bc = const_p.tile([P, NF], f32)
        nc.sync.dma_start(fwb_bc, fwb_d[:][None, :].to_broadcast((P, NF)))
        mdb_bc = const_p.tile([P, F], f32)
        nc.sync.dma_start(mdb_bc, mdb_d[:][None, :].to_broadcast((P, F)))


        # ---- dense heads, replicated across all 128 partitions ---------
        # lhsT columns all equal w[n], so every psum partition holds the
        # same dense result; softmax runs redundantly per partition and
        # fbw/w1 land pre-broadcast with no DRAM bounce.
        fbw_bc = []
        w1rep = []
        for n in range(NS):
            wrep = bc_p.tile([P, KO, P], cdt, tag="wrep")
            nc.vector.tensor_copy(
                wrep, wt[:, :, n][:, :, None].to_broadcast((P, KO, P)))
            ps_l = ps_small.tile([P, NF], f32, tag="sm")
            for ko in range(KO):
                nc.tensor.matmul(ps_l, lhsT=wrep[:, ko, :], rhs=fww[:, ko, :],
                                 start=(ko == 0), stop=(ko == KO - 1))
            logits = small_p.tile([P, NF], f32, tag="logits")
            nc.vector.tensor_tensor(logits, ps_l, fwb_bc, aa)
            # softmax denominator cancels through the per-tap demod norm
            # (filters/||filters|| is invariant to any per-sample scalar),
            # so raw exp(logits) works as mixing weights.
            fb = bc_p.tile([P, NF], f32, tag="fbw_bc")
            nc.scalar.activation(fb, logits, ACT.Exp)
            fbw_bc.append(fb)

            ps_sc = ps_small.tile([P, F], f32, tag="sm")
            for ko in range(KO):
                nc.tensor.matmul(ps_sc, lhsT=wrep[:, ko, :], rhs=mdw[:, ko, :],
                                 start=(ko == 0), stop=(ko == KO - 1))
            w1 = bc_p.tile([P, F], cdt, tag="w1_bc")
            nc.vector.scalar_tensor_tensor(w1, ps_sc, 1.0, mdb_bc, aa, aa)
            w1r = bc_p.tile([P, TAPS, F], cdt, tag="w1rep")
            nc.vector.tensor_copy(
                w1r, w1[:, None, :].to_broadcast((P, TAPS, F)))
            w1rep.append(w1r)

        # psum for HAM warmup matmuls
        ps_warm = ps_small.tile([P, 512], f32, tag="warm")
        ones_sb = const_p.tile([P, P], f32)
        nc.vector.memset(ones_sb, 1.0)
        ident = const_p.tile([P, P], cdt)
        nc.sync.dma_start(ident, ident_d[:, :])

        # ---- per-sample: pad x, mix+modulate+demod filters, conv -------
        for n in range(NS):
            # host-pre-padded input in flat coords, [c=128, 66*66+slack]
            xpt = []
            for ct in range(CT):
                xp = xpad_p.tile([P, XLPAD], cdt, tag="xpad")
                nc.scalar.dma_start(xp, x_d[n, ct, :, :])
                xpt.append(xp)

            # mix bank with fbw, then modulate by w1.
            # Sample 0 is on the critical path: accumulate fbw_b * bank_b on
            # the otherwise-idle PE as matmuls with diagonal fbw_b*I weights
            # (f32 PSUM accumulate). Sample 1 mixes on DVE under sample 0's
            # conv.
            acc = []
            q0 = []
            if n == 0:
                fbwI = []
                for b in range(NF):
                    dg = diag_p.tile([P, P], cdt, tag="diag")
                    nc.vector.tensor_scalar_mul(dg, ident,
                                                fbw_bc[n][:, b:b + 1])
                    fbwI.append(dg)
                    if WARM_MM:
                        for _ in range(2):
                            nc.tensor.matmul(ps_warm[:, :P], lhsT=dg, rhs=dg,
                                             start=True, stop=True)
                for ct in range(CT):
                    bks = []
                    for b in range(NF):
                        bk = bank_p.tile([P, TAPS, F], cdt, tag=f"bk{ct}")
                        nc.sync.dma_start(bk,
                                          bank_d[b, ct * P:(ct + 1) * P, :, :])
                        bks.append(bk)
                        if ct == 0 and WARM_MM:
                            for _ in range(WARM_MM):
                                nc.tensor.matmul(ps_warm,
                                                 lhsT=bk[:, 0, 0:P],
                                                 rhs=bk[:, 0:2, :],
                                                 start=True, stop=True)
                    a = mix_p.tile([P, TAPS, F], cdt, tag="acc")
                    af = a.rearrange("p t f -> p (t f)")
                    scr = tmp_p.tile([P, TAPS, F], cdt, tag="tmp")
                    scrf = scr.rearrange("p t f -> p (t f)")
                    w1f = w1rep[n].rearrange("p t f -> p (t f)")
                    qt = small_p.tile([P, TAPS], f32, tag="q")
                    for ci, (off, csz) in enumerate(MIX_CHUNKS):
                        ps = ps_conv.tile([P, 512], f32, tag="cv")
                        for b in range(NF):
                            bf = bks[b].rearrange("p t f -> p (t f)")
                            nc.tensor.matmul(ps[:, :csz], lhsT=fbwI[b],
                                             rhs=bf[:, off:off + csz],
                                             start=(b == 0),
                                             stop=(b == NF - 1))
                        nc.vector.tensor_tensor(af[:, off:off + csz],
                                                ps[:, :csz],
                                                w1f[:, off:off + csz], mm)
                        nc.scalar.activation(scrf[:, off:off + csz],
                                             af[:, off:off + csz], ACT.Square)
                        if ci == 3:
                            # taps 0-7 reduce overlaps the final mix chunk
                            nc.vector.tensor_reduce(
                                qt[:, 0:8], scr[:, 0:8, :],
                                axis=mybir.AxisListType.X, op=aa)
                    nc.vector.tensor_reduce(qt[:, 8:9], scr[:, 8:9, :],
                                            axis=mybir.AxisListType.X, op=aa)
                    q0.append(qt)
                    acc.append(a)
            else:
                for ct in range(CT):
                    a = mix_p.tile([P, TAPS, F], cdt, tag="acc")
                    for b in range(NF):
                        bk = bank_p.tile([P, TAPS, F], cdt, tag=f"bk{ct}")
                        nc.sync.dma_start(bk,
                                          bank_d[b, ct * P:(ct + 1) * P, :, :])
                        if b == 0:
                            nc.vector.tensor_scalar_mul(a, bk,
                                                        fbw_bc[n][:, 0:1])
                        else:
                            t = tmp_p.tile([P, TAPS, F], cdt, tag="tmp")
                            if b in (5, 7):
                                nc.scalar.activation(
                                    t, bk, ACT.Copy,
                                    scale=fbw_bc[n][:, b:b + 1])
                            else:
                                nc.vector.tensor_scalar_mul(
                                    t, bk, fbw_bc[n][:, b:b + 1])
                            nc.vector.tensor_tensor(a, a, t, aa)
                    nc.vector.tensor_tensor(a, a, w1rep[n], mm)
                    acc.append(a)

            # per-tap demod norm over (f, c)
            if n == 0:
                q = q0
            else:
                q = []
                for ct in range(CT):
                    scr = tmp_p.tile([P, TAPS, F], cdt, tag="tmp")
                    nc.scalar.activation(scr, acc[ct], ACT.Square)
                    qt = small_p.tile([P, TAPS], f32, tag="q")
                    nc.vector.tensor_reduce(qt, scr,
                                            axis=mybir.AxisListType.X, op=aa)
                    q.append(qt)
            qs = small_p.tile([P, TAPS], f32, tag="qs")
            nc.vector.tensor_tensor(qs, q[0], q[1], aa)
            ps_nrm = ps_small.tile([P, TAPS], f32, tag="sm")
            nc.tensor.matmul(ps_nrm, lhsT=ones_sb, rhs=qs,
                             start=True, stop=True)
            nall = small_p.tile([P, TAPS], f32, tag="nall")
            nc.vector.tensor_scalar_max(nall, ps_nrm, EPS)
            sq = small_p.tile([P, TAPS], f32, tag="sq")
            nc.scalar.activation(sq, nall, ACT.Sqrt)
            ninv = small_p.tile([P, TAPS], f32, tag="ninv")
            nc.vector.reciprocal(ninv, sq)
            if n == 0 and WARM_MM:
                for _ in range(4):
                    nc.tensor.matmul(ps_warm[:, :TAPS], lhsT=ones_sb,
                                     rhs=qs, start=True, stop=True)

            filt = []
            for ct in range(CT):
                fl = filtb_p.tile([P, TAPS, F], cdt, tag="filt")
                for tp in range(TAPS):
                    if tp % 2 == 0:
                        nc.vector.tensor_scalar_mul(
                            fl[:, tp, :], acc[ct][:, tp, :],
                            ninv[:, tp:tp + 1])
                    else:
                        nc.scalar.activation(fl[:, tp, :], acc[ct][:, tp, :],
                                             ACT.Copy,
                                             scale=ninv[:, tp:tp + 1])
                filt.append(fl)

            # conv in padded-flat coords: rhs slices are contiguous so the
            # PE streams at full rate; cols 64,65 of each row are garbage
            # and dropped by the strided output DMA.
            for ft in range(FT):
                osb = out_p.tile([P, OPIX], f32, tag="osb")
                for off, sz in CHUNKS:
                    ps = ps_conv.tile([P, 512], f32, tag="cv")
                    k = 0
                    for ct in range(CT):
                        for kh in range(KK):
                            for kw in range(KK):
                                rhs = xpt[ct][:, off + kh * WP + kw:
                                              off + kh * WP + kw + sz]
                                nc.tensor.matmul(
                                    ps[:, :sz],
                                    lhsT=filt[ct][:, kh * KK + kw,
                                                  ft * P:(ft + 1) * P],
                                    rhs=rhs,
                                    start=(k == 0), stop=(k == 2 * TAPS - 1))
                                k += 1
                    nc.scalar.activation(osb[:, off:off + sz], ps[:, :sz],
                                         ACT.Copy)
                ov = osb.rearrange("p (h w) -> p h w", w=WP)
                HS = 38
                nc.scalar.dma_start(out_d[n, ft * P:(ft + 1) * P, 0:HS, :],
                                    ov[:, 0:HS, 0:W])
                nc.scalar.dma_start(out_d[n, ft * P:(ft + 1) * P, HS:H, :],
                                    ov[:, HS:H, 0:W])

    nc.compile()
    return nc


def kernel(x, w, filter_bank, dense_fw_w, dense_fw_b, dense_mod_w, dense_mod_b):
    global LAST
    x = np.ascontiguousarray(np.asarray(x, dtype=np.float32))
    w = np.ascontiguousarray(np.asarray(w, dtype=np.float32))
    xdt = np.float32
    if USE_BF16:
        import ml_dtypes
        xdt = ml_dtypes.bfloat16
    NB = x.shape[0]
    xp_all = np.zeros((NB, CT, P, XLPAD), dtype=xdt)
    xv = xp_all[:, :, :, :XL].reshape(NB, CT, P, HP, WP)
    xv[:, :, :, 1:H + 1, 1:W + 1] = x.reshape(NB, CT, P, H, W)
    fb = np.asarray(filter_bank, dtype=np.float32)
    # [b, f, c, kh, kw] -> [b, c, (kh kw), f]
    bank_t = np.ascontiguousarray(
        np.transpose(fb, (0, 2, 3, 4, 1)).reshape(NF, C, TAPS, F))
    if USE_BF16:
        import ml_dtypes
        bank_t = bank_t.astype(ml_dtypes.bfloat16)

    trace = os.environ.get("KERNEL_TRACE", "") == "1"
    if trace:
        import types

        import concourse.bass_utils as bu
        bu.upload_artifacts = lambda tmpdir: tmpdir
        if "antenv.axon_hooks" not in sys.modules:
            from trn_agent_boot.trn_boot import _ntff_profile_via_ctypes
            hook = _ntff_profile_via_ctypes("/opt/axon/libaxon_pjrt.so")
            mod = types.ModuleType("antenv.axon_hooks")
            mod.get_axon_ntff_profile_hook = lambda: hook
            sys.modules["antenv.axon_hooks"] = mod

    nc = _build()
    in_maps = []
    for core in range(N_CORES):
        sl = slice(core * NS, (core + 1) * NS)
        w_t = np.ascontiguousarray(
            w[sl].reshape(NS, KO, P).transpose(2, 1, 0))
        ident = np.eye(P, dtype=xdt)
        in_maps.append({
            "ident": ident,
            "xp": np.ascontiguousarray(xp_all[sl]),
            "wv_t": w_t,
            "bank_t": bank_t,
            "fw_w": np.ascontiguousarray(np.asarray(dense_fw_w, np.float32).astype(xdt)),
            "fw_b": np.ascontiguousarray(np.asarray(dense_fw_b, np.float32)),
            "md_w": np.ascontiguousarray(np.asarray(dense_mod_w, np.float32).astype(xdt)),
            "md_b": np.ascontiguousarray(np.asarray(dense_mod_b, np.float32)),
        })
    kwargs = {}
    if trace:
        import tempfile
        base = os.environ.get("KERNEL_TRACE_DIR", "/tmp/ktrace")
        os.makedirs(base, exist_ok=True)
        tdir = tempfile.mkdtemp(dir=base)
        print(f"trace dir: {tdir}", flush=True)
        kwargs = dict(trace=True, tmpdir=tdir)
    LAST = run_bass_kernel_spmd(nc, in_maps, core_ids=list(range(N_CORES)),
                                **kwargs)
    return np.concatenate([LAST.results[i]["out"] for i in range(N_CORES)],
                          axis=0)
